# revision 7
# baseline (speedup 1.0000x reference)
"""Trainium2 Bass kernel for nn_ImageSparseAttention.

Self-contained: shards FULL inputs across 8 NeuronCores (1 batch per core),
runs a fused Bass/Tile kernel per core, gathers the FULL output.

Reference computation (per batch b):
    q  = image @ Wq.T + bq                     (IB, D)
    k  = text  @ Wk.T + bk                     (S, D)
    v  = text  @ Wv.T + bv                     (S, D)
    sparse = topk-scatter(softmax(band) @ Wb2s.T + bb2s)   (IB, S)  batch-invariant
    kk = sparse @ k                            (IB, D)
    Y  = kk.T @ Wb2s.T                         (D, S)    [reassociated chain]
    out2T = Y.T-contraction: out2T[s,i] = sum_d Y[d,s] qT[d,i]/sqrt(D) + bb2s[s]
    E  = exp(out2T)            (no max-sub: logits are small ~N(0,1))
    out[i,d] = sum_s E[s,i] v[s,d] / sum_s E[s,i]  + bv[d]
"""
import math
import os
import sys

import numpy as np

for _p in (
    "/root/.axon_site",
    "/root/.axon_site/_ro/trn_rl_repo",
    "/root/.axon_site/_ro/pypackages",
    "/opt/trn_rl_repo",
):
    if os.path.isdir(_p) and _p not in sys.path:
        sys.path.append(_p)

import concourse.bacc as bacc
import concourse.mybir as mybir
import concourse.tile as tile
from concourse.bass_utils import run_bass_kernel_spmd

dt = mybir.dt
AFT = mybir.ActivationFunctionType

B, IB, S, D = 8, 1024, 4096, 512
WINDOW = 1
SPARSITY = 2
NNZ = S // SPARSITY + 2 * WINDOW

N_CORES = 8
SCALE = 1.0 / math.sqrt(D)

ST = S // 128    # 32 s-tiles
IT = IB // 128   # 8 i-tiles
DC = D // 128    # 4 d/c chunks
F32R = dt.float32r
F32 = dt.float32


def build_nc(debug_dump=False):
    nc = bacc.Bacc("TRN2", target_bir_lowering=False, debug=False,
                   num_devices=N_CORES)

    # ---- DRAM I/O (per-core) ----
    textT_d = nc.dram_tensor("textT", [D, S], F32R, kind="ExternalInput")
    imgT_d = nc.dram_tensor("imageT", [D, IB], F32R, kind="ExternalInput")
    wq_d = nc.dram_tensor("wqT", [D, D], F32R, kind="ExternalInput")
    wk_d = nc.dram_tensor("wkT", [D, D], F32R, kind="ExternalInput")
    wv_d = nc.dram_tensor("wvT", [D, D], F32R, kind="ExternalInput")
    spT_d = nc.dram_tensor("sparseT", [S, IB], F32R, kind="ExternalInput")
    wbT_d = nc.dram_tensor("wb2sT", [IB, S], F32R, kind="ExternalInput")
    ones_d = nc.dram_tensor("ones", [128, 2], F32R, kind="ExternalInput")
    bk_d = nc.dram_tensor("bk_bcast", [128, D], F32, kind="ExternalInput")
    bv_d = nc.dram_tensor("bv_bcast", [128, D], F32, kind="ExternalInput")
    bq_d = nc.dram_tensor("bq_col", [128, DC], F32, kind="ExternalInput")
    bb_d = nc.dram_tensor("bb2s_col", [128, ST], F32, kind="ExternalInput")
    out_d = nc.dram_tensor("out", [IB, D], F32, kind="ExternalOutput")
    if debug_dump:
        qT_dbg = nc.dram_tensor("qT_dbg", [128, DC * IB], F32, kind="ExternalOutput")
        kk_dbg = nc.dram_tensor("kk_dbg", [128, IT * D], F32, kind="ExternalOutput")
        y_dbg = nc.dram_tensor("y_dbg", [128, DC * S], F32, kind="ExternalOutput")
        e_dbg = nc.dram_tensor("e_dbg", [2 * ST, 128, 512], F32, kind="ExternalOutput")
        rs_dbg = nc.dram_tensor("rs_dbg", [2, 128, 8], F32, kind="ExternalOutput")

    with tile.TileContext(nc) as tc:
        with tc.tile_pool(name="consts", bufs=1) as consts, \
             tc.tile_pool(name="qT", bufs=1) as qT_pool, \
             tc.tile_pool(name="kk", bufs=1) as kk_pool, \
             tc.tile_pool(name="v", bufs=1) as v_pool:

            ones_sb = consts.tile([128, 2], F32R)
            bk_sb = consts.tile([128, D], F32)
            bv_sb = consts.tile([128, D], F32)
            bq_sb = consts.tile([128, DC], F32)
            bb_sb = consts.tile([128, ST], F32)
            nc.sync.dma_start(ones_sb[:], ones_d[:])
            nc.sync.dma_start(bk_sb[:], bk_d[:])
            nc.sync.dma_start(bv_sb[:], bv_d[:])
            nc.sync.dma_start(bq_sb[:], bq_d[:])
            nc.sync.dma_start(bb_sb[:], bb_d[:])

            qT_sb = qT_pool.tile([128, DC * IB], F32R)   # [p, dc*IB + i]
            kk_sb = kk_pool.tile([128, IT * D], F32R)    # [p, it*D + d]
            v_sb = v_pool.tile([128, ST * D], F32R)      # [p, st*D + d]

            # ---------- Phase Q: qT[d,i] = (Wq @ image.T + bq)/sqrt(D) ----
            with tc.tile_pool(name="wq", bufs=1) as wq_pool, \
                 tc.tile_pool(name="imgT", bufs=1) as img_pool, \
                 tc.tile_pool(name="psQ", bufs=2, space="PSUM") as psQ:
                wq_sb = wq_pool.tile([128, DC * D], F32R)   # [p, c*D + j]
                img_sb = img_pool.tile([128, DC * IB], F32R)  # [p, c*IB + i]
                nc.sync.dma_start(
                    wq_sb[:].rearrange("p (c j) -> p c j", c=DC),
                    wq_d[:].rearrange("(c p) j -> p c j", p=128))
                nc.sync.dma_start(
                    img_sb[:].rearrange("p (c i) -> p c i", c=DC),
                    imgT_d[:].rearrange("(c p) i -> p c i", p=128))
                for dc in range(DC):
                    for ih in range(2):
                        pq = psQ.tile([128, 512], F32)
                        for c in range(DC):
                            nc.tensor.matmul(
                                pq[:],
                                wq_sb[:, c * D + dc * 128:c * D + (dc + 1) * 128],
                                img_sb[:, c * IB + ih * 512:c * IB + (ih + 1) * 512],
                                start=(c == 0), stop=(c == DC - 1))
                        # (psum + bq) * 1/sqrt(D)  -> f32r
                        nc.vector.tensor_scalar(
                            qT_sb[:, dc * IB + ih * 512:dc * IB + (ih + 1) * 512],
                            pq[:], bq_sb[:, dc:dc + 1], SCALE,
                            mybir.AluOpType.add, mybir.AluOpType.mult)

            # ---------- Phase A: k,v = text @ W.T + b (k into stream, v resident)
            with tc.tile_pool(name="kbuf", bufs=1) as k_pool:
                k_sb = k_pool.tile([128, ST * D], F32R)   # [p, st*D + d]
                with tc.tile_pool(name="wkv", bufs=1) as wkv_pool, \
                     tc.tile_pool(name="ttile", bufs=2) as tt_pool, \
                     tc.tile_pool(name="psA", bufs=2, space="PSUM") as psA:
                    wk_sb = wkv_pool.tile([128, DC * D], F32R)
                    wv_sb = wkv_pool.tile([128, DC * D], F32R)
                    nc.sync.dma_start(
                        wk_sb[:].rearrange("p (c j) -> p c j", c=DC),
                        wk_d[:].rearrange("(c p) j -> p c j", p=128))
                    nc.sync.dma_start(
                        wv_sb[:].rearrange("p (c j) -> p c j", c=DC),
                        wv_d[:].rearrange("(c p) j -> p c j", p=128))
                    for sc in range(8):  # chunks of 512 along s
                        tt = tt_pool.tile([128, DC * 512], F32R)  # [p, c*512+j]
                        eng = nc.sync if sc % 2 == 0 else nc.gpsimd
                        eng.dma_start(
                            tt[:].rearrange("p (c j) -> p c j", c=DC),
                            textT_d[:, sc * 512:(sc + 1) * 512]
                            .rearrange("(c p) j -> p c j", p=128))
                        for sl in range(4):
                            st = sc * 4 + sl
                            pk = psA.tile([128, D], F32)
                            pv = psA.tile([128, D], F32)
                            for c in range(DC):
                                lhs = tt[:, c * 512 + sl * 128:c * 512 + (sl + 1) * 128]
                                nc.tensor.matmul(
                                    pk[:], lhs, wk_sb[:, c * D:(c + 1) * D],
                                    start=(c == 0), stop=(c == DC - 1))
                            for c in range(DC):
                                lhs = tt[:, c * 512 + sl * 128:c * 512 + (sl + 1) * 128]
                                nc.tensor.matmul(
                                    pv[:], lhs, wv_sb[:, c * D:(c + 1) * D],
                                    start=(c == 0), stop=(c == DC - 1))
                            # k = psum + bk  (DVE), v = copy (ACT)
                            nc.vector.tensor_tensor(
                                k_sb[:, st * D:(st + 1) * D], pk[:], bk_sb[:],
                                mybir.AluOpType.add)
                            nc.scalar.copy(v_sb[:, st * D:(st + 1) * D], pv[:])

                # ---------- Phase B: kk[i,d] = sum_s sparseT[s,i] k[s,d] ----
                with tc.tile_pool(name="spT", bufs=4) as sp_pool, \
                     tc.tile_pool(name="psB", bufs=8, space="PSUM") as psB:
                    pkk = [psB.tile([128, D], F32, name=f"pkk{i}", tag="pkk") for i in range(IT)]
                    for st in range(ST):
                        sp = sp_pool.tile([128, IB], F32R)
                        eng = nc.sync if st % 2 == 0 else nc.gpsimd
                        eng.dma_start(sp[:], spT_d[st * 128:(st + 1) * 128, :])
                        for it in range(IT):
                            nc.tensor.matmul(
                                pkk[it][:],
                                sp[:, it * 128:(it + 1) * 128],
                                k_sb[:, st * D:(st + 1) * D],
                                start=(st == 0), stop=(st == ST - 1))
                    for it in range(IT):
                        nc.vector.tensor_copy(kk_sb[:, it * D:(it + 1) * D], pkk[it][:])

            # ---------- Phase C: Y[d,s] = sum_i kk[i,d] Wb2sT[i,s] ----
            with tc.tile_pool(name="Y", bufs=1) as y_pool:
                Y_sb = y_pool.tile([128, DC * S], F32R)   # [p, dc*S + s]
                with tc.tile_pool(name="wb", bufs=6) as wb_pool, \
                     tc.tile_pool(name="psC", bufs=8, space="PSUM") as psC:
                    for sc in range(8):
                        py = [psC.tile([128, 512], F32, name=f"py{i}", tag="py") for i in range(DC)]
                        for ic in range(IT):
                            wb = wb_pool.tile([128, 512], F32R)
                            eng = nc.sync if ic % 2 == 0 else nc.gpsimd
                            eng.dma_start(
                                wb[:],
                                wbT_d[ic * 128:(ic + 1) * 128,
                                      sc * 512:(sc + 1) * 512])
                            for dtl in range(DC):
                                nc.tensor.matmul(
                                    py[dtl][:],
                                    kk_sb[:, ic * D + dtl * 128:ic * D + (dtl + 1) * 128],
                                    wb[:],
                                    start=(ic == 0), stop=(ic == IT - 1))
                        for dtl in range(DC):
                            nc.scalar.copy(
                                Y_sb[:, dtl * S + sc * 512:dtl * S + (sc + 1) * 512],
                                py[dtl][:])
                    if debug_dump:
                        nc.sync.dma_start(y_dbg[:], Y_sb[:].bitcast(F32))
                        nc.sync.dma_start(qT_dbg[:], qT_sb[:].bitcast(F32))
                        nc.sync.dma_start(kk_dbg[:], kk_sb[:].bitcast(F32))

                # ---------- Phase D: softmax(out2T) @ v, fused ----
                with tc.tile_pool(name="E", bufs=3) as e_pool, \
                     tc.tile_pool(name="psD", bufs=2, space="PSUM") as psD, \
                     tc.tile_pool(name="psOut", bufs=4, space="PSUM") as psOut, \
                     tc.tile_pool(name="psRs", bufs=1, space="PSUM") as psRs, \
                     tc.tile_pool(name="fin", bufs=4) as fin_pool, \
                     tc.tile_pool(name="rcp", bufs=2) as rcp_pool:
                    for h in range(2):
                        pout = [psOut.tile([128, D], F32, name=f"pout{i}", tag="pout") for i in range(4)]
                        prs = psRs.tile([128, 8], F32)
                        for st in range(ST):
                            p2 = psD.tile([128, 512], F32)
                            for dc in range(DC):
                                nc.tensor.matmul(
                                    p2[:],
                                    Y_sb[:, dc * S + st * 128:dc * S + (st + 1) * 128],
                                    qT_sb[:, dc * IB + h * 512:dc * IB + (h + 1) * 512],
                                    start=(dc == 0), stop=(dc == DC - 1))
                            et = e_pool.tile([128, 512], F32R)
                            nc.scalar.activation(
                                et[:], p2[:], AFT.Exp,
                                bias=bb_sb[:, st:st + 1], scale=1.0)
                            if debug_dump:
                                nc.sync.dma_start(e_dbg[h * ST + st, :, :],
                                                  et[:].bitcast(F32))
                            for t in range(4):
                                nc.tensor.matmul(
                                    pout[t][:],
                                    et[:, t * 128:(t + 1) * 128],
                                    v_sb[:, st * D:(st + 1) * D],
                                    start=(st == 0), stop=(st == ST - 1))
                                nc.tensor.matmul(
                                    prs[:, 2 * t:2 * t + 2],
                                    et[:, t * 128:(t + 1) * 128],
                                    ones_sb[:, 0:2],
                                    start=(st == 0), stop=(st == ST - 1))
                        if debug_dump:
                            rs_cp = rcp_pool.tile([128, 8], F32, name="rs_cp")
                            nc.vector.tensor_copy(rs_cp[:], prs[:])
                            nc.sync.dma_start(rs_dbg[h, :, :], rs_cp[:])
                        rcp = rcp_pool.tile([128, 4], F32)
                        for t in range(4):
                            nc.vector.reciprocal(rcp[:, t:t + 1], prs[:, 2 * t:2 * t + 1])
                        for t in range(4):
                            tmp = fin_pool.tile([128, D], F32)
                            outf = fin_pool.tile([128, D], F32)
                            nc.scalar.activation(
                                tmp[:], pout[t][:], AFT.Copy,
                                scale=rcp[:, t:t + 1])
                            nc.vector.tensor_tensor(
                                outf[:], tmp[:], bv_sb[:], mybir.AluOpType.add)
                            it_glob = h * 4 + t
                            nc.sync.dma_start(
                                out_d[it_glob * 128:(it_glob + 1) * 128, :], outf[:])

    nc.compile()
    return nc


def host_sparse(Wb2s, bb2s):
    """Batch-invariant (IB,S) sparse matrix, replicating reference ops on CPU."""
    import jax
    import jax.numpy as jnp
    cpu = jax.local_devices(backend="cpu")[0]
    with jax.default_device(cpu):
        i = jnp.arange(IB)[:, None]
        j = jnp.arange(IB)[None, :]
        mask = ((j >= i - WINDOW) & (j <= i + WINDOW)).astype(jnp.float32)
        aw = jax.nn.softmax(mask, axis=-1)
        aw = aw @ jnp.asarray(Wb2s).T + jnp.asarray(bb2s)
        topk_scores, topk_idx = jax.lax.top_k(aw, NNZ)
        sparse = jnp.zeros((IB, S), aw.dtype).at[
            jnp.arange(IB)[:, None], topk_idx].set(topk_scores)
        return np.asarray(sparse)


_NC_CACHE = None


def _get_nc():
    global _NC_CACHE
    if _NC_CACHE is None:
        _NC_CACHE = build_nc()
    return _NC_CACHE


def make_in_maps(text_feature, image_feature, Wq, bq, Wk, bk, Wv, bv, Wb2s, bb2s):
    f32 = np.float32
    text = np.asarray(text_feature, f32)
    image = np.asarray(image_feature, f32)
    sparse = host_sparse(np.asarray(Wb2s, f32), np.asarray(bb2s, f32))

    shared = {
        "wqT": np.ascontiguousarray(np.asarray(Wq, f32).T),
        "wkT": np.ascontiguousarray(np.asarray(Wk, f32).T),
        "wvT": np.ascontiguousarray(np.asarray(Wv, f32).T),
        "sparseT": np.ascontiguousarray(sparse.T),
        "wb2sT": np.ascontiguousarray(np.asarray(Wb2s, f32).T),
        "ones": np.ones((128, 2), f32),
        "bk_bcast": np.ascontiguousarray(
            np.broadcast_to(np.asarray(bk, f32), (128, D))),
        "bv_bcast": np.ascontiguousarray(
            np.broadcast_to(np.asarray(bv, f32), (128, D))),
        "bq_col": np.ascontiguousarray(np.asarray(bq, f32).reshape(DC, 128).T),
        "bb2s_col": np.ascontiguousarray(np.asarray(bb2s, f32).reshape(ST, 128).T),
    }
    in_maps = []
    for b in range(N_CORES):
        m = dict(shared)
        m["textT"] = np.ascontiguousarray(text[b].T)
        m["imageT"] = np.ascontiguousarray(image[b].T)
        in_maps.append(m)
    return in_maps


def kernel(**inputs):
    nc = _get_nc()
    in_maps = make_in_maps(**inputs)
    res = run_bass_kernel_spmd(nc, in_maps, list(range(N_CORES)))
    out = np.stack([res.results[b]["out"] for b in range(N_CORES)], axis=0)
    return out.astype(np.float32)
